# revision 13
# baseline (speedup 1.0000x reference)
"""Multi-head self-attention (B=2, T=2048, C=1024, H=16) on 8 TRN2 NeuronCores.

Sharding: tensor-parallel over heads. Core m owns heads (2m, 2m+1):
  - computes qkv^T = (Wqkv_shard^T) @ x^T for its 2 heads (contraction-major
    layouts; host pre-transposes x so no on-chip transposes of x are needed).
    x and Wqkv stream in bf16 (halves HBM traffic; PE rate identical).
  - causal attention for its 2 heads (both batches), flash-style with
    blockwise exp (no max-subtraction: scores are O(1) here) and a
    ones-column in v that produces the softmax denominator for free
  - partial output projection partial_m = values_m @ Wo[rows of heads m],
    written to HBM in bf16.
Host sums the 8 partials in fp32 and adds bias bo.

Pipeline structure (per (b, i-chunk)):
  - scores for both heads land in one 2-bank PSUM tile -> single fused exp
  - on causal-diagonal blocks only columns >= 128*r are computed and only the
    leading 128-col triangle is masked (DVE); off-diagonal blocks need none
  - pv matmuls run with a 2-step lookahead behind scores so the PE never
    waits on the scalar engine's exp
  - normalization (fast reciprocal of the denominator row + PE row-broadcast)
    and the output projection of the PREVIOUS i-chunk are injected into the
    current i-chunk's attention steps to fill PE gaps
"""

import numpy as np

import concourse.bass as bass
import concourse.bacc as bacc
import concourse.mybir as mybir
import concourse.tile as tile
from concourse.bass_utils import run_bass_kernel_spmd

B, T, C = 2, 2048, 1024
H, HS = 16, 64
N_CORES = 8
HPC = H // N_CORES            # heads per core = 2
R = B * T                     # 4096 rows total
IC_W = 512                    # i-chunk width (query cols per block)
JT_W = 128                    # j-tile width (key rows per block)
N_IC = T // IC_W              # 4 i-chunks per batch
N_JT = T // JT_W              # 16 j-tiles per batch
F32 = mybir.dt.float32
F32R = mybir.dt.float32r
BF16 = mybir.dt.bfloat16


def _build(causal: bool, reps: int = 1):
    nc = bacc.Bacc("TRN2", target_bir_lowering=False, debug=False,
                   num_devices=N_CORES)

    xt_d = nc.dram_tensor("xt", [C, R], BF16, kind="ExternalInput")
    w3_d = nc.dram_tensor("w3", [C, 3 * 128], BF16, kind="ExternalInput")
    b3_d = nc.dram_tensor("b3", [3, 128, 1], F32, kind="ExternalInput")
    wo_d = nc.dram_tensor("wo", [128, C], F32R, kind="ExternalInput")
    tril_d = nc.dram_tensor("tril", [128, 2 * JT_W], F32R, kind="ExternalInput")
    idn_d = nc.dram_tensor("idn", [128, 64], F32R, kind="ExternalInput")
    out_d = nc.dram_tensor("part", [R, C], BF16, kind="ExternalOutput")

    with tile.TileContext(nc) as tc:
        with (
            tc.tile_pool(name="const", bufs=1) as cpool,
            tc.tile_pool(name="qkv", bufs=1) as qkvpool,
            tc.tile_pool(name="xt", bufs=16) as xtpool,
            tc.tile_pool(name="pt", bufs=4) as ptpool,
            tc.tile_pool(name="ot", bufs=2) as otpool,
            tc.tile_pool(name="osb", bufs=4) as opool,
            tc.tile_pool(name="rcp", bufs=2) as rpool,
            tc.tile_pool(name="ps_mm", bufs=2, space="PSUM") as ps_mm,
            tc.tile_pool(name="ps_s", bufs=2, space="PSUM") as ps_s,
            tc.tile_pool(name="ps_o", bufs=1, space="PSUM") as ps_o,
        ):
            for rep in range(reps):
                # ---- xt prefetch: one [128, 1024] bf16 DMA per c-chunk per
                # pair of i-chunks (2 KB per partition line). The very first
                # pair is split into half-partition DMAs so all 16 DMA
                # engines work on it in parallel (shortest time-to-first-mm).
                pf = {}

                def prefetch(b, pair, fine=False):
                    base = T * b + 1024 * pair
                    tiles = []
                    for ct in range(8):
                        t_ = xtpool.tile([128, 1024], BF16)
                        src = xt_d.ap()[128 * ct:128 * (ct + 1),
                                        base:base + 1024]
                        if fine:
                            nc.sync.dma_start(t_[0:64, :], src[0:64, :])
                            nc.sync.dma_start(t_[64:128, :], src[64:128, :])
                        else:
                            nc.sync.dma_start(t_[:], src)
                        tiles.append(t_)
                    pf[(b, pair)] = tiles

                # ---- constants (w3 first; wo is not needed until the first
                # projection, so it goes last) ----
                w3_sb = []
                for ct in range(8):
                    t_ = cpool.tile([128, 384], BF16, tag=f"w3_{ct}")
                    nc.sync.dma_start(t_[:], w3_d.ap()[128 * ct:128 * (ct + 1), :])
                    w3_sb.append(t_)
                    if ct == 0:
                        prefetch(0, 0, fine=True)
                bias_sb = []
                for n in range(3):
                    t_ = cpool.tile([128, 1], F32, tag=f"b3_{n}")
                    nc.sync.dma_start(t_[:], b3_d.ap()[n])
                    bias_sb.append(t_)
                tril_sb = cpool.tile([128, 2 * JT_W], F32R, tag="tril")
                nc.sync.dma_start(tril_sb[:], tril_d.ap()[:])
                idn_sb = cpool.tile([128, 64], F32R, tag="idn")
                nc.sync.dma_start(idn_sb[:], idn_d.ap()[:])
                prefetch(0, 1)
                wo_sb = cpool.tile([128, C], F32R, tag="wo")
                nc.sync.dma_start(wo_sb[:], wo_d.ap()[:])

                # persistent qkv^T (transposed layouts, heads packed 2-up)
                qt2b = [qkvpool.tile([128, T], F32R, tag=f"qt2_{b_}",
                                     name=f"qt2_{b_}") for b_ in range(B)]
                kt2b = [qkvpool.tile([128, T], F32R, tag=f"kt2_{b_}",
                                     name=f"kt2_{b_}") for b_ in range(B)]
                vt2b = [qkvpool.tile([128, T], F32R, tag=f"vt2_{b_}",
                                     name=f"vt2_{b_}") for b_ in range(B)]
                # v in natural [key, dim] layout per (b, h), + ones column
                vn_sb = {}
                for b in range(B):
                    for h in range(HPC):
                        t_ = qkvpool.tile([128, N_JT * (HS + 1)], F32R,
                                          name=f"vn_{rep}_{b}_{h}",
                                          tag=f"vn_{b}_{h}")
                        vn_sb[(b, h)] = t_
                        # ones column at offset 64 of each 65-wide group
                        # (memset of float 1.0 fails the ISA check on f32r;
                        # write the fp32 bit pattern through a uint32 view)
                        nc.vector.memset(
                            t_[:].rearrange("p (j c) -> p j c", c=HS + 1)
                            [:, :, HS:HS + 1].bitcast(mybir.dt.uint32),
                            0x3F800000)

                def qkv_phase(b, icl):
                    """project x -> q^T,k^T,v^T for i-chunk icl of batch b,
                    and build natural-v tiles for this icl's 4 j-tiles."""
                    c0 = IC_W * (icl % 2)
                    xts = [t_[:, c0:c0 + IC_W] for t_ in pf[(b, icl // 2)]]
                    for n, dst in enumerate((qt2b[b], kt2b[b], vt2b[b])):
                        ps = ps_mm.tile([128, IC_W], F32, tag="mm")
                        for ct in range(8):
                            nc.tensor.matmul(
                                ps[:],
                                w3_sb[ct][:, 128 * n:128 * (n + 1)],
                                xts[ct],
                                start=(ct == 0), stop=(ct == 7))
                        nc.vector.tensor_scalar_add(
                            dst[:, IC_W * icl:IC_W * (icl + 1)], ps[:],
                            bias_sb[n][:])
                    # natural-layout v for j-tiles 4*icl .. 4*icl+3
                    for jt in range(4 * icl, 4 * icl + 4):
                        j0 = JT_W * jt
                        for h in range(HPC):
                            psv = ps_mm.tile([128, IC_W], F32R, tag="mm")
                            nc.tensor.transpose(
                                psv[:, 0:HS],
                                vt2b[b][64 * h:64 * (h + 1), j0:j0 + JT_W],
                                idn_sb[64 * h:64 * (h + 1), 0:64])
                            nc.vector.tensor_copy(
                                vn_sb[(b, h)][:, 65 * jt:65 * jt + HS],
                                psv[:, 0:HS])

                def norm_phase(b, icl, pso, ot):
                    """ot[:,q] = pso[h][0:64,q] / pso[h][64,q] for both heads."""
                    for h in range(HPC):
                        csr = rpool.tile([1, IC_W], F32R, tag="rcp")
                        nc.scalar.activation(
                            csr[:], pso[h][64:65, :],
                            mybir.ActivationFunctionType.Copy)
                        pscb = ps_mm.tile([128, IC_W], F32, tag="mm")
                        nc.tensor.matmul(
                            pscb[0:64, :], tril_sb[0:1, 0:64], csr[:],
                            start=True, stop=True)
                        rcb = rpool.tile([64, IC_W], F32, tag="rcb")
                        nc.vector.reciprocal_approx_fast(
                            rcb[:], pscb[0:64, :])
                        nc.vector.tensor_mul(
                            ot[64 * h:64 * (h + 1), :], pso[h][0:64, :],
                            rcb[:])

                def proj_items(b, icl, ot):
                    """output projection for i-chunk: 4 it x 2 oc matmuls,
                    returned as closures to inject into attention steps."""
                    items = []
                    for it in range(4):
                        def mk(it):
                            def run():
                                osb = opool.tile([128, C], BF16, tag="osb")
                                for oc in range(2):
                                    psp = ps_mm.tile([128, IC_W], F32,
                                                     tag="mm")
                                    nc.tensor.matmul(
                                        psp[:],
                                        ot[:, 128 * it:128 * (it + 1)],
                                        wo_sb[:, IC_W * oc:IC_W * (oc + 1)],
                                        start=True, stop=True)
                                    nc.vector.tensor_copy(
                                        osb[:, IC_W * oc:IC_W * (oc + 1)],
                                        psp[:])
                                r0 = T * b + IC_W * icl + 128 * it
                                nc.sync.dma_start(
                                    out_d.ap()[r0:r0 + 128, :], osb[:])
                            return run
                        items.append(mk(it))
                    return items

                def att_phase(b, icl, inject):
                    """attention for i-chunk icl of batch b; `inject` is a
                    list of closures (prev norm+proj work) drained into the
                    step loop to fill PE gaps."""
                    i0 = IC_W * icl
                    njt = 4 * icl + 4 if causal else N_JT
                    pso = [ps_o.tile([128, IC_W], F32, tag=f"o{h_}",
                                     name=f"pso_{h_}") for h_ in range(HPC)]
                    pts = {}
                    c0s = {}

                    def score(k):
                        jt = k
                        j0 = JT_W * jt
                        r_ = jt - 4 * icl if causal else -1
                        c0 = 128 * r_ if r_ > 0 else 0
                        c0s[k] = c0
                        ps2 = ps_s.tile([128, 2 * IC_W], F32, tag="s")
                        for h in range(HPC):
                            h0 = 64 * h
                            nc.tensor.matmul(
                                ps2[:, IC_W * h + c0:IC_W * (h + 1)],
                                kt2b[b][h0:h0 + 64, j0:j0 + JT_W],
                                qt2b[b][h0:h0 + 64, i0 + c0:i0 + IC_W],
                                start=True, stop=True,
                                tile_position=(h0, 0),
                                skip_group_check=True)
                        pt = ptpool.tile([128, 2 * IC_W], F32R, tag="pt")
                        pts[k] = pt
                        pv2 = ps2[:].rearrange("p (h w) -> p h w", h=2)
                        ptv = pt[:].rearrange("p (h w) -> p h w", h=2)
                        nc.scalar.activation(
                            ptv[:, :, c0:], pv2[:, :, c0:],
                            mybir.ActivationFunctionType.Exp)
                        if causal and r_ >= 0:
                            # only the leading 128-col triangle needs masking
                            nc.vector.tensor_mul(
                                ptv[:, :, c0:c0 + JT_W],
                                ptv[:, :, c0:c0 + JT_W],
                                tril_sb[:].rearrange(
                                    "p (h w) -> p h w", h=2))

                    def pv(k):
                        jt = k
                        c0 = c0s[k]
                        ptv = pts[k][:].rearrange("p (h w) -> p h w", h=2)
                        for h in range(HPC):
                            nc.tensor.matmul(
                                pso[h][0:65, c0:],
                                vn_sb[(b, h)][:, 65 * jt:65 * (jt + 1)],
                                ptv[:, h, c0:],
                                start=(k == 0), stop=(k == njt - 1),
                                tile_position=(0, 0), skip_group_check=True)
                        del pts[k]

                    for k in range(njt + 2):
                        if k < njt:
                            score(k)
                        if k == 1 and inject:
                            inject.pop(0)()  # norm of prev (frees pso bufs)
                        elif k >= 2:
                            if inject:
                                inject.pop(0)()
                            pv(k - 2)
                    while inject:
                        inject.pop(0)()
                    return pso

                # ---- main schedule ----
                prev = None  # (b, icl, pso)
                for b in range(B):
                    for icl in range(N_IC):
                        if b == 0 and icl == 2:
                            prefetch(1, 0)
                        elif b == 1 and icl == 0:
                            prefetch(1, 1)
                        qkv_phase(b, icl)
                        inject = []
                        if prev is not None:
                            pb, picl, ppso = prev
                            pot = otpool.tile([128, IC_W], F32R, tag="ot")
                            inject.append(
                                lambda pb=pb, picl=picl, ppso=ppso, pot=pot:
                                norm_phase(pb, picl, ppso, pot))
                            inject.extend(proj_items(pb, picl, pot))
                        pso = att_phase(b, icl, inject)
                        prev = (b, icl, pso)
                # final i-chunk's norm + projection
                pb, picl, ppso = prev
                pot = otpool.tile([128, IC_W], F32R, tag="ot")
                norm_phase(pb, picl, ppso, pot)
                for it_run in proj_items(pb, picl, pot):
                    it_run()
    nc.compile()
    return nc


_PROGS = {}


def _get_prog(causal: bool, reps: int = 1):
    key = (causal, reps)
    if key not in _PROGS:
        _PROGS[key] = _build(causal, reps)
    return _PROGS[key]


def _bf16(a):
    return a.astype(mybir.dt.np(BF16))


def _prep_inputs(x, Wqkv, bqkv, Wo):
    """Per-core input maps (host-side sharding)."""
    x = np.asarray(x, dtype=np.float32)
    Wqkv = np.asarray(Wqkv, dtype=np.float32)
    bqkv = np.asarray(bqkv, dtype=np.float32)
    Wo = np.asarray(Wo, dtype=np.float32)

    xt = _bf16(np.ascontiguousarray(x.reshape(R, C).T))  # [C, R] bf16

    # inclusive lower-triangle tile for the diagonal 128x128 blocks
    jl = np.arange(JT_W)[:, None]
    il = np.arange(JT_W)[None, :]
    tril1 = (jl <= il).astype(np.float32)
    tril = np.concatenate([tril1, tril1], axis=1)  # [128, 256] for both heads
    idn = np.tile(np.eye(64, dtype=np.float32), (2, 1))

    in_maps = []
    scale = 1.0 / np.sqrt(np.float32(HS))
    for m in range(N_CORES):
        h0, h1 = HPC * m, HPC * m + 1
        cols = {}
        for name, off, sc in (("q", 0, scale), ("k", HS, 1.0), ("v", 2 * HS, 1.0)):
            blk = [Wqkv[:, 192 * h + off:192 * h + off + HS] * sc
                   for h in (h0, h1)]
            bb = [bqkv[192 * h + off:192 * h + off + HS] * sc for h in (h0, h1)]
            cols[name] = (np.concatenate(blk, axis=1),
                          np.concatenate(bb))
        w3 = np.concatenate([cols["q"][0], cols["k"][0], cols["v"][0]], axis=1)
        b3 = np.stack([cols["q"][1], cols["k"][1], cols["v"][1]])[..., None]
        wo = Wo[128 * m:128 * (m + 1), :]
        in_maps.append({
            "xt": xt,
            "w3": np.ascontiguousarray(_bf16(w3)),
            "b3": np.ascontiguousarray(b3.astype(np.float32)),
            "wo": np.ascontiguousarray(wo.astype(np.float32)),
            "tril": tril,
            "idn": idn,
        })
    return in_maps


class _Runner:
    """Cached shard_map runner for the SPMD NEFF (avoids re-jit per call)."""

    def __init__(self, nc):
        import jax
        from jax.sharding import Mesh, PartitionSpec
        from jax.experimental.shard_map import shard_map
        from concourse import bass2jax

        bass2jax.install_neuronx_cc_hook()

        part_name = (nc.partition_id_tensor.name
                     if nc.partition_id_tensor else None)
        in_names, out_names, out_avals, zero_outs = [], [], [], []
        for alloc in nc.m.functions[0].allocations:
            if not isinstance(alloc, mybir.MemoryLocationSet):
                continue
            name = alloc.memorylocations[0].name
            if alloc.kind == "ExternalInput":
                if name != part_name:
                    in_names.append(name)
            elif alloc.kind == "ExternalOutput":
                out_names.append(name)
                shape = tuple(alloc.tensor_shape)
                dtype = mybir.dt.np(alloc.dtype)
                out_avals.append(jax.core.ShapedArray(shape, dtype))
                zero_outs.append(np.zeros(shape, dtype))
        self.in_names, self.out_names = in_names, out_names
        self.zero_outs = zero_outs
        n_params, n_outs = len(in_names), len(out_names)
        all_in_names = tuple(in_names) + tuple(out_names)
        if part_name is not None:
            all_in_names = all_in_names + (part_name,)

        def _exec(args, outs):
            operands = list(args) + list(outs)
            if part_name is not None:
                operands.append(bass2jax.partition_id_tensor())
            return bass2jax._bass_exec_p.bind(
                *operands,
                out_avals=tuple(out_avals),
                in_names=all_in_names,
                out_names=tuple(out_names),
                lowering_input_output_aliases=(),
                sim_require_finite=True,
                sim_require_nnan=True,
                nc=nc)

        def _body(*args):
            ins, outs = args[:n_params], list(args[n_params:])
            return tuple(_exec(ins, outs))

        devices = jax.devices()[:N_CORES]
        mesh = Mesh(np.asarray(devices), ("core",))
        donate = tuple(range(n_params, n_params + n_outs))
        self._fn = jax.jit(
            shard_map(_body, mesh=mesh,
                      in_specs=(PartitionSpec("core"),) * (n_params + n_outs),
                      out_specs=(PartitionSpec("core"),) * n_outs,
                      check_rep=False),
            donate_argnums=donate, keep_unused=True)

    def __call__(self, in_maps):
        concat_in = [
            np.concatenate([in_maps[c][k] for c in range(N_CORES)], axis=0)
            for k in self.in_names]
        concat_zero = [
            np.zeros((N_CORES * z.shape[0], *z.shape[1:]), z.dtype)
            for z in self.zero_outs]
        out = self._fn(*concat_in, *concat_zero)
        return [
            {k: np.asarray(out[i]).reshape(N_CORES, *self.zero_outs[i].shape)[c]
             for i, k in enumerate(self.out_names)}
            for c in range(N_CORES)]


_RUNNERS = {}


def _get_runner(causal: bool, reps: int = 1):
    key = (causal, reps)
    if key not in _RUNNERS:
        _RUNNERS[key] = _Runner(_get_prog(causal, reps))
    return _RUNNERS[key]


def kernel(x, Wqkv, bqkv, Wo, bo, mask):
    causal = bool(np.asarray(mask).item()) if not isinstance(mask, (int, bool)) else bool(mask)
    runner = _get_runner(causal)
    in_maps = _prep_inputs(x, Wqkv, bqkv, Wo)
    results = runner(in_maps)
    acc = np.zeros((R, C), dtype=np.float32)
    for m in range(N_CORES):
        acc += results[m]["part"].astype(np.float32)
    acc += np.asarray(bo, dtype=np.float32)[None, :]
    return acc.reshape(B, T, C)


# revision 18
# speedup vs baseline: 1.0380x; 1.0380x over previous
"""Multi-head self-attention (B=2, T=2048, C=1024, H=16) on 8 TRN2 NeuronCores.

Sharding: tensor-parallel over heads. Core m owns heads (2m, 2m+1):
  - computes qkv^T = (Wqkv_shard^T) @ x^T for its 2 heads (contraction-major
    layouts; host pre-transposes x so no on-chip transposes of x are needed).
    x and Wqkv stream in bf16 (halves HBM traffic; PE rate identical).
  - causal attention for its 2 heads (both batches), flash-style with
    blockwise exp (no max-subtraction: scores are O(1) here) and a
    ones-column in v that produces the softmax denominator for free
  - partial output projection partial_m = values_m @ Wo[rows of heads m],
    written to HBM in bf16.
Host sums the 8 partials in fp32 and adds bias bo.

Pipeline structure (per (b, i-chunk)):
  - scores for both heads land in one 2-bank PSUM tile -> single fused exp
  - on causal-diagonal blocks only columns >= 128*r are computed and only the
    leading 128-col triangle is masked (DVE); off-diagonal blocks need none
  - pv matmuls run with a 2-step lookahead behind scores so the PE never
    waits on the scalar engine's exp
  - normalization (fast reciprocal of the denominator row + PE row-broadcast)
    and the output projection of the PREVIOUS i-chunk are injected into the
    current i-chunk's attention steps to fill PE gaps
"""

import numpy as np

import concourse.bass as bass
import concourse.bacc as bacc
import concourse.mybir as mybir
import concourse.tile as tile
from concourse.bass_utils import run_bass_kernel_spmd

B, T, C = 2, 2048, 1024
H, HS = 16, 64
N_CORES = 8
HPC = H // N_CORES            # heads per core = 2
R = B * T                     # 4096 rows total
IC_W = 512                    # i-chunk width (query cols per block)
JT_W = 128                    # j-tile width (key rows per block)
N_IC = T // IC_W              # 4 i-chunks per batch
N_JT = T // JT_W              # 16 j-tiles per batch
F32 = mybir.dt.float32
F32R = mybir.dt.float32r
BF16 = mybir.dt.bfloat16


def _build(causal: bool, reps: int = 1):
    nc = bacc.Bacc("TRN2", target_bir_lowering=False, debug=False,
                   num_devices=N_CORES)

    xt_d = nc.dram_tensor("xt", [C, R], BF16, kind="ExternalInput")
    w3_d = nc.dram_tensor("w3", [C, 3 * 128], BF16, kind="ExternalInput")
    b3_d = nc.dram_tensor("b3", [3, 128, 1], F32, kind="ExternalInput")
    wo_d = nc.dram_tensor("wo", [128, C], F32R, kind="ExternalInput")
    tril_d = nc.dram_tensor("tril", [128, 2 * JT_W], F32R, kind="ExternalInput")
    idn_d = nc.dram_tensor("idn", [128, 64], F32R, kind="ExternalInput")
    out_d = nc.dram_tensor("part", [R, C], BF16, kind="ExternalOutput")

    with tile.TileContext(nc) as tc:
        with (
            tc.tile_pool(name="const", bufs=1) as cpool,
            tc.tile_pool(name="qkv", bufs=1) as qkvpool,
            tc.tile_pool(name="xt", bufs=16) as xtpool,
            tc.tile_pool(name="pt", bufs=4) as ptpool,
            tc.tile_pool(name="ot", bufs=2) as otpool,
            tc.tile_pool(name="osb", bufs=4) as opool,
            tc.tile_pool(name="rcp", bufs=2) as rpool,
            tc.tile_pool(name="ps_mm", bufs=2, space="PSUM") as ps_mm,
            tc.tile_pool(name="ps_s", bufs=2, space="PSUM") as ps_s,
            tc.tile_pool(name="ps_o", bufs=1, space="PSUM") as ps_o,
        ):
            for rep in range(reps):
                # ---- xt prefetch: one [128, 1024] bf16 DMA per c-chunk per
                # pair of i-chunks (2 KB per partition line). The very first
                # pair is split into half-partition DMAs so all 16 DMA
                # engines work on it in parallel (shortest time-to-first-mm).
                pf = {}

                def prefetch(b, pair):
                    base = T * b + 1024 * pair
                    tiles = []
                    for ct in range(8):
                        t_ = xtpool.tile([128, 1024], BF16, name="xt_t", tag="xt")
                        nc.sync.dma_start(
                            t_[:], xt_d.ap()[128 * ct:128 * (ct + 1),
                                             base:base + 1024])
                        tiles.append(t_)
                    pf[(b, pair)] = tiles

                # ---- startup: w3 chunk 0 + the first i-chunk's x columns
                # land first, split across both HWDGE queues (sync + scalar)
                # and across partition halves so all DMA engines pull in
                # parallel. wo is not needed until the first projection, so
                # it goes last. ----
                w3_sb = [cpool.tile([128, 384], BF16, tag=f"w3_{ct}",
                                    name=f"w3sb_{ct}")
                         for ct in range(8)]
                nc.sync.dma_start(w3_sb[0][:], w3_d.ap()[0:128, :])
                pf00 = [xtpool.tile([128, 1024], BF16, name=f"pf00_{i_}", tag="xt")
                        for i_ in range(8)]
                pf[(0, 0)] = pf00
                for ct in range(8):
                    src = xt_d.ap()[128 * ct:128 * (ct + 1), 0:IC_W]
                    nc.sync.dma_start(pf00[ct][0:64, 0:IC_W], src[0:64, :])
                    nc.scalar.dma_start(pf00[ct][64:128, 0:IC_W],
                                        src[64:128, :])
                for ct in range(1, 8):
                    nc.scalar.dma_start(
                        w3_sb[ct][:], w3_d.ap()[128 * ct:128 * (ct + 1), :])
                bias_sb = []
                for n in range(3):
                    t_ = cpool.tile([128, 1], F32, tag=f"b3_{n}")
                    nc.sync.dma_start(t_[:], b3_d.ap()[n])
                    bias_sb.append(t_)
                tril_sb = cpool.tile([128, 2 * JT_W], F32R, tag="tril")
                nc.sync.dma_start(tril_sb[:], tril_d.ap()[:])
                idn_sb = cpool.tile([128, 64], F32R, tag="idn")
                nc.sync.dma_start(idn_sb[:], idn_d.ap()[:])
                # second i-chunk's columns of pair (0, 0)
                for ct in range(8):
                    nc.sync.dma_start(
                        pf00[ct][:, IC_W:1024],
                        xt_d.ap()[128 * ct:128 * (ct + 1), IC_W:1024])
                prefetch(0, 1)
                wo_sb = cpool.tile([128, C], F32R, tag="wo")
                nc.scalar.dma_start(wo_sb[:], wo_d.ap()[:])

                # persistent qkv^T (transposed layouts, heads packed 2-up)
                qt2b = [qkvpool.tile([128, T], F32R, tag=f"qt2_{b_}",
                                     name=f"qt2_{b_}") for b_ in range(B)]
                kt2b = [qkvpool.tile([128, T], F32R, tag=f"kt2_{b_}",
                                     name=f"kt2_{b_}") for b_ in range(B)]
                vt2b = [qkvpool.tile([128, T], F32R, tag=f"vt2_{b_}",
                                     name=f"vt2_{b_}") for b_ in range(B)]
                # v in natural [key, dim] layout per (b, h), + ones column
                vn_sb = {}
                for b in range(B):
                    for h in range(HPC):
                        t_ = qkvpool.tile([128, N_JT * (HS + 1)], F32R,
                                          name=f"vn_{rep}_{b}_{h}",
                                          tag=f"vn_{b}_{h}")
                        vn_sb[(b, h)] = t_
                        # ones column at offset 64 of each 65-wide group
                        # (memset of float 1.0 fails the ISA check on f32r;
                        # write the fp32 bit pattern through a uint32 view)
                        nc.vector.memset(
                            t_[:].rearrange("p (j c) -> p j c", c=HS + 1)
                            [:, :, HS:HS + 1].bitcast(mybir.dt.uint32),
                            0x3F800000)

                def qkv_phase(b, icl):
                    """project x -> q^T,k^T,v^T for i-chunk icl of batch b,
                    and build natural-v tiles for this icl's 4 j-tiles."""
                    c0 = IC_W * (icl % 2)
                    xts = [t_[:, c0:c0 + IC_W] for t_ in pf[(b, icl // 2)]]
                    for n, dst in enumerate((qt2b[b], kt2b[b], vt2b[b])):
                        ps = ps_mm.tile([128, IC_W], F32, tag="mm")
                        for ct in range(8):
                            nc.tensor.matmul(
                                ps[:],
                                w3_sb[ct][:, 128 * n:128 * (n + 1)],
                                xts[ct],
                                start=(ct == 0), stop=(ct == 7))
                        nc.vector.tensor_scalar_add(
                            dst[:, IC_W * icl:IC_W * (icl + 1)], ps[:],
                            bias_sb[n][:])
                    # natural-layout v for j-tiles 4*icl .. 4*icl+3
                    for jt in range(4 * icl, 4 * icl + 4):
                        j0 = JT_W * jt
                        for h in range(HPC):
                            psv = ps_mm.tile([128, IC_W], F32R, tag="mm")
                            nc.tensor.transpose(
                                psv[:, 0:HS],
                                vt2b[b][64 * h:64 * (h + 1), j0:j0 + JT_W],
                                idn_sb[64 * h:64 * (h + 1), 0:64])
                            nc.vector.tensor_copy(
                                vn_sb[(b, h)][:, 65 * jt:65 * jt + HS],
                                psv[:, 0:HS])

                def norm_phase(b, icl, pso, ot):
                    """ot[:,q] = pso[h][0:64,q] / pso[h][64,q] for both heads."""
                    for h in range(HPC):
                        csr = rpool.tile([1, IC_W], F32R, tag="rcp")
                        nc.vector.tensor_copy(csr[:], pso[h][64:65, :])
                        pscb = ps_mm.tile([128, IC_W], F32, tag="mm")
                        nc.tensor.matmul(
                            pscb[0:64, :], tril_sb[0:1, 0:64], csr[:],
                            start=True, stop=True)
                        rcb = rpool.tile([64, IC_W], F32, tag="rcb")
                        nc.vector.reciprocal_approx_fast(
                            rcb[:], pscb[0:64, :])
                        nc.vector.tensor_mul(
                            ot[64 * h:64 * (h + 1), :], pso[h][0:64, :],
                            rcb[:])

                def proj_items(b, icl, ot):
                    """output projection for i-chunk: 4 it x 2 oc matmuls,
                    returned as closures to inject into attention steps."""
                    items = []
                    for it in range(4):
                        def mk(it):
                            def run():
                                osb = opool.tile([128, C], BF16, tag="osb")
                                for oc in range(2):
                                    psp = ps_mm.tile([128, IC_W], F32,
                                                     tag="mm")
                                    nc.tensor.matmul(
                                        psp[:],
                                        ot[:, 128 * it:128 * (it + 1)],
                                        wo_sb[:, IC_W * oc:IC_W * (oc + 1)],
                                        start=True, stop=True)
                                    nc.vector.tensor_copy(
                                        osb[:, IC_W * oc:IC_W * (oc + 1)],
                                        psp[:])
                                r0 = T * b + IC_W * icl + 128 * it
                                nc.sync.dma_start(
                                    out_d.ap()[r0:r0 + 128, :], osb[:])
                            return run
                        items.append(mk(it))
                    return items

                def att_phase(b, icl, inject):
                    """attention for i-chunk icl of batch b; `inject` is a
                    list of closures (prev norm+proj work) drained into the
                    step loop to fill PE gaps."""
                    i0 = IC_W * icl
                    njt = 4 * icl + 4 if causal else N_JT
                    pso = [ps_o.tile([128, IC_W], F32, tag=f"o{h_}",
                                     name=f"pso_{h_}") for h_ in range(HPC)]
                    pts = {}
                    c0s = {}

                    def score(k):
                        jt = k
                        j0 = JT_W * jt
                        r_ = jt - 4 * icl if causal else -1
                        c0 = 128 * r_ if r_ > 0 else 0
                        c0s[k] = c0
                        ps2 = ps_s.tile([128, 2 * IC_W], F32, tag="s")
                        for h in range(HPC):
                            h0 = 64 * h
                            nc.tensor.matmul(
                                ps2[:, IC_W * h + c0:IC_W * (h + 1)],
                                kt2b[b][h0:h0 + 64, j0:j0 + JT_W],
                                qt2b[b][h0:h0 + 64, i0 + c0:i0 + IC_W],
                                start=True, stop=True,
                                tile_position=(h0, 0),
                                skip_group_check=True)
                        pt = ptpool.tile([128, 2 * IC_W], F32R, tag="pt")
                        pts[k] = pt
                        pv2 = ps2[:].rearrange("p (h w) -> p h w", h=2)
                        ptv = pt[:].rearrange("p (h w) -> p h w", h=2)
                        nc.scalar.activation(
                            ptv[:, :, c0:], pv2[:, :, c0:],
                            mybir.ActivationFunctionType.Exp)
                        if causal and r_ >= 0:
                            # only the leading 128-col triangle needs masking
                            nc.vector.tensor_mul(
                                ptv[:, :, c0:c0 + JT_W],
                                ptv[:, :, c0:c0 + JT_W],
                                tril_sb[:].rearrange(
                                    "p (h w) -> p h w", h=2))

                    def pv(k):
                        jt = k
                        c0 = c0s[k]
                        ptv = pts[k][:].rearrange("p (h w) -> p h w", h=2)
                        for h in range(HPC):
                            nc.tensor.matmul(
                                pso[h][0:65, c0:],
                                vn_sb[(b, h)][:, 65 * jt:65 * (jt + 1)],
                                ptv[:, h, c0:],
                                start=(k == 0), stop=(k == njt - 1),
                                tile_position=(0, 0), skip_group_check=True)
                        del pts[k]

                    for k in range(njt + 2):
                        if k < njt:
                            score(k)
                        if k == 1 and inject:
                            inject.pop(0)()  # norm of prev (frees pso bufs)
                        elif k >= 2:
                            pv(k - 2)
                            if inject:
                                inject.pop(0)()
                    while inject:
                        inject.pop(0)()
                    return pso

                # ---- main schedule ----
                prev = None  # (b, icl, pso)
                for b in range(B):
                    for icl in range(N_IC):
                        if b == 0 and icl == 2:
                            prefetch(1, 0)
                        elif b == 1 and icl == 0:
                            prefetch(1, 1)
                        qkv_phase(b, icl)
                        inject = []
                        if prev is not None:
                            pb, picl, ppso = prev
                            pot = otpool.tile([128, IC_W], F32R, tag="ot")
                            inject.append(
                                lambda pb=pb, picl=picl, ppso=ppso, pot=pot:
                                norm_phase(pb, picl, ppso, pot))
                            inject.extend(proj_items(pb, picl, pot))
                        pso = att_phase(b, icl, inject)
                        prev = (b, icl, pso)
                # final i-chunk's norm + projection
                pb, picl, ppso = prev
                pot = otpool.tile([128, IC_W], F32R, tag="ot")
                norm_phase(pb, picl, ppso, pot)
                for it_run in proj_items(pb, picl, pot):
                    it_run()
    nc.compile()
    return nc


_PROGS = {}


def _get_prog(causal: bool, reps: int = 1):
    key = (causal, reps)
    if key not in _PROGS:
        _PROGS[key] = _build(causal, reps)
    return _PROGS[key]


def _bf16(a):
    return a.astype(mybir.dt.np(BF16))


def _prep_inputs(x, Wqkv, bqkv, Wo):
    """Per-core input maps (host-side sharding)."""
    x = np.asarray(x, dtype=np.float32)
    Wqkv = np.asarray(Wqkv, dtype=np.float32)
    bqkv = np.asarray(bqkv, dtype=np.float32)
    Wo = np.asarray(Wo, dtype=np.float32)

    xt = _bf16(np.ascontiguousarray(x.reshape(R, C).T))  # [C, R] bf16

    # inclusive lower-triangle tile for the diagonal 128x128 blocks
    jl = np.arange(JT_W)[:, None]
    il = np.arange(JT_W)[None, :]
    tril1 = (jl <= il).astype(np.float32)
    tril = np.concatenate([tril1, tril1], axis=1)  # [128, 256] for both heads
    idn = np.tile(np.eye(64, dtype=np.float32), (2, 1))

    in_maps = []
    scale = 1.0 / np.sqrt(np.float32(HS))
    for m in range(N_CORES):
        h0, h1 = HPC * m, HPC * m + 1
        cols = {}
        for name, off, sc in (("q", 0, scale), ("k", HS, 1.0), ("v", 2 * HS, 1.0)):
            blk = [Wqkv[:, 192 * h + off:192 * h + off + HS] * sc
                   for h in (h0, h1)]
            bb = [bqkv[192 * h + off:192 * h + off + HS] * sc for h in (h0, h1)]
            cols[name] = (np.concatenate(blk, axis=1),
                          np.concatenate(bb))
        w3 = np.concatenate([cols["q"][0], cols["k"][0], cols["v"][0]], axis=1)
        b3 = np.stack([cols["q"][1], cols["k"][1], cols["v"][1]])[..., None]
        wo = Wo[128 * m:128 * (m + 1), :]
        in_maps.append({
            "xt": xt,
            "w3": np.ascontiguousarray(_bf16(w3)),
            "b3": np.ascontiguousarray(b3.astype(np.float32)),
            "wo": np.ascontiguousarray(wo.astype(np.float32)),
            "tril": tril,
            "idn": idn,
        })
    return in_maps


class _Runner:
    """Cached shard_map runner for the SPMD NEFF (avoids re-jit per call)."""

    def __init__(self, nc):
        import jax
        from jax.sharding import Mesh, PartitionSpec
        from jax.experimental.shard_map import shard_map
        from concourse import bass2jax

        bass2jax.install_neuronx_cc_hook()

        part_name = (nc.partition_id_tensor.name
                     if nc.partition_id_tensor else None)
        in_names, out_names, out_avals, zero_outs = [], [], [], []
        for alloc in nc.m.functions[0].allocations:
            if not isinstance(alloc, mybir.MemoryLocationSet):
                continue
            name = alloc.memorylocations[0].name
            if alloc.kind == "ExternalInput":
                if name != part_name:
                    in_names.append(name)
            elif alloc.kind == "ExternalOutput":
                out_names.append(name)
                shape = tuple(alloc.tensor_shape)
                dtype = mybir.dt.np(alloc.dtype)
                out_avals.append(jax.core.ShapedArray(shape, dtype))
                zero_outs.append(np.zeros(shape, dtype))
        self.in_names, self.out_names = in_names, out_names
        self.zero_outs = zero_outs
        n_params, n_outs = len(in_names), len(out_names)
        all_in_names = tuple(in_names) + tuple(out_names)
        if part_name is not None:
            all_in_names = all_in_names + (part_name,)

        def _exec(args, outs):
            operands = list(args) + list(outs)
            if part_name is not None:
                operands.append(bass2jax.partition_id_tensor())
            return bass2jax._bass_exec_p.bind(
                *operands,
                out_avals=tuple(out_avals),
                in_names=all_in_names,
                out_names=tuple(out_names),
                lowering_input_output_aliases=(),
                sim_require_finite=True,
                sim_require_nnan=True,
                nc=nc)

        def _body(*args):
            ins, outs = args[:n_params], list(args[n_params:])
            return tuple(_exec(ins, outs))

        devices = jax.devices()[:N_CORES]
        mesh = Mesh(np.asarray(devices), ("core",))
        donate = tuple(range(n_params, n_params + n_outs))
        self._fn = jax.jit(
            shard_map(_body, mesh=mesh,
                      in_specs=(PartitionSpec("core"),) * (n_params + n_outs),
                      out_specs=(PartitionSpec("core"),) * n_outs,
                      check_rep=False),
            donate_argnums=donate, keep_unused=True)

    def __call__(self, in_maps):
        concat_in = [
            np.concatenate([in_maps[c][k] for c in range(N_CORES)], axis=0)
            for k in self.in_names]
        concat_zero = [
            np.zeros((N_CORES * z.shape[0], *z.shape[1:]), z.dtype)
            for z in self.zero_outs]
        out = self._fn(*concat_in, *concat_zero)
        return [
            {k: np.asarray(out[i]).reshape(N_CORES, *self.zero_outs[i].shape)[c]
             for i, k in enumerate(self.out_names)}
            for c in range(N_CORES)]


_RUNNERS = {}


def _get_runner(causal: bool, reps: int = 1):
    key = (causal, reps)
    if key not in _RUNNERS:
        _RUNNERS[key] = _Runner(_get_prog(causal, reps))
    return _RUNNERS[key]


def kernel(x, Wqkv, bqkv, Wo, bo, mask):
    causal = bool(np.asarray(mask).item()) if not isinstance(mask, (int, bool)) else bool(mask)
    runner = _get_runner(causal)
    in_maps = _prep_inputs(x, Wqkv, bqkv, Wo)
    results = runner(in_maps)
    acc = np.zeros((R, C), dtype=np.float32)
    for m in range(N_CORES):
        acc += results[m]["part"].astype(np.float32)
    acc += np.asarray(bo, dtype=np.float32)[None, :]
    return acc.reshape(B, T, C)
